# revision 23
# baseline (speedup 1.0000x reference)
"""Trainium2 Bass kernel for CAttention:
    k      = einsum('bcit,i->bct', x, alpha)
    scores = einsum('bct,ts,bds->bcd', k, Wc, k)
    att    = softmax(scores, axis=-1)
    out    = einsum('bci,bint->bcnt', att, x)

Sharding: data-parallel over batch B=64 across 8 NeuronCores (8 batches/core).

Per-core layout (per batch b):
    X SBUF tile [128, 8192]: partition p = j*8 + d  (j in [0,16) = n-chunk,
    d in [0,8) = channel), free q = n2*64 + t with n = j*128 + n2.

    k-path : s[(j,d),t] = sum_n2 alpha[j*128+n2] * X  (DVE mul + strided reduce)
             kT[t,d]    = sum_(j,d') s * sel          (PE, s_t as stationary)
    scores : V = Wc @ kT (PE, WcT const); scores = kT.T @ V (PE)
    softmax: unnormalized exp on ACT (accum row-sum); 1/sum replicated via PE;
             normalization folded into the PSUM-evacuation scale.
    mix    : block-diag(e^T) [128,128] stationary, one full-width PE pass
    out    : ACT evacuates PSUM -> SBUF with per-partition 1/sum scale, DMA out

Emission is software-pipelined (phase A of batch b+1 before phase B of
batch b) so the DVE stream's big multiply/reduce never stalls behind the
previous batch's cross-engine softmax chain.
"""

import sys

for _p in ("/opt/trn_rl_repo",):
    if _p not in sys.path:
        sys.path.insert(0, _p)

import numpy as np

B, C, N, T = 64, 8, 2048, 64
NCORES = 8
BS = B // NCORES          # batches per core
J = 16                    # n-chunks on partitions
N2 = N // J               # 128, n-extent in free dim
P = J * C                 # 128 partitions
F = N2 * T                # 8192 free elems
QW = 512                  # mix matmul free width (one PSUM bank)

_PROGRAM_CACHE = {}


def _build_program():
    from contextlib import ExitStack

    import concourse.bacc as bacc
    from concourse import mybir, tile

    fp32 = mybir.dt.float32
    nc = bacc.Bacc("TRN2", target_bir_lowering=False, debug=False)

    xs = nc.dram_tensor("xs", [BS, C, N, T], fp32, kind="ExternalInput").ap()
    wcT = nc.dram_tensor("wcT", [T, T], fp32, kind="ExternalInput").ap()
    ac = nc.dram_tensor("ac", [P, N2], fp32, kind="ExternalInput").ap()
    sel = nc.dram_tensor("sel", [P, C], fp32, kind="ExternalInput").ap()
    id8 = nc.dram_tensor("id8", [C, C], fp32, kind="ExternalInput").ap()
    rep = nc.dram_tensor("rep", [C, P], fp32, kind="ExternalInput").ap()
    mask = nc.dram_tensor("mask", [P, P], fp32, kind="ExternalInput").ap()
    out = nc.dram_tensor("out", [BS, C, N, T], fp32, kind="ExternalOutput").ap()

    Exp = mybir.ActivationFunctionType.Exp
    Copy = mybir.ActivationFunctionType.Copy
    AX = mybir.AxisListType.X
    ADD = mybir.AluOpType.add
    MULT = mybir.AluOpType.mult

    with tile.TileContext(nc) as tc, ExitStack() as ctx:
        cpool = ctx.enter_context(tc.tile_pool(name="const", bufs=1))
        xpool = ctx.enter_context(tc.tile_pool(name="x", bufs=3))
        scrpool = ctx.enter_context(tc.tile_pool(name="scr", bufs=1))
        opool = ctx.enter_context(tc.tile_pool(name="o", bufs=3))
        spool = ctx.enter_context(tc.tile_pool(name="small", bufs=2))
        bdpool = ctx.enter_context(tc.tile_pool(name="bd", bufs=2))
        mixp = ctx.enter_context(tc.tile_pool(name="mixp", bufs=4, space="PSUM"))
        psmall = ctx.enter_context(tc.tile_pool(name="psmall", bufs=3, space="PSUM"))

        wcT_t = cpool.tile([T, T], fp32)
        nc.sync.dma_start(wcT_t[:], wcT)
        ac_t = cpool.tile([P, N2], fp32)
        nc.sync.dma_start(ac_t[:], ac)
        sel_t = cpool.tile([P, C], fp32)
        nc.sync.dma_start(sel_t[:], sel)
        id8_t = cpool.tile([C, C], fp32)
        nc.sync.dma_start(id8_t[:], id8)
        rep_t = cpool.tile([C, P], fp32)
        nc.sync.dma_start(rep_t[:], rep)
        mask_t = cpool.tile([P, P], fp32)
        nc.sync.dma_start(mask_t[:], mask)

        def phase_a(b):
            """DMA-in + alpha-weighted partial reduction (big DVE work)."""
            X = xpool.tile([P, F], fp32, tag="X")
            # split the input stream across two descriptor paths (HWDGE sync
            # ring + SWDGE gpsimd ring) so each SDMA engine has two queues of
            # outstanding HBM reads (reads are latency-bound at one queue)
            xsrc = xs[b].rearrange("d (j n2) t -> j d (n2 t)", j=J)
            nc.sync.dma_start(X[: P // 2, :], xsrc[: J // 2])
            nc.gpsimd.dma_start(X[P // 2 :, :], xsrc[J // 2 :])
            # alpha-weighted product into a dedicated scratch (freed as soon
            # as the kT matmul has read the tree result in scr[:, :T])
            scr = scrpool.tile([P, F], fp32, tag="scr")
            nc.vector.tensor_tensor(
                out=scr[:].rearrange("p (n2 t) -> p n2 t", t=T),
                in0=X[:].rearrange("p (n2 t) -> p n2 t", t=T),
                in1=ac_t[:].rearrange("p (x n2) -> p n2 x", x=1).to_broadcast(
                    [P, N2, T]
                ),
                op=MULT,
            )
            # contiguous in-place tree reduction over n2 (halving along the
            # n2-major free axis; each step adds two contiguous blocks)
            w = F // 2
            while w >= T:
                nc.vector.tensor_tensor(
                    out=scr[:, :w], in0=scr[:, :w], in1=scr[:, w : 2 * w], op=ADD
                )
                w //= 2
            return X, scr

        def phase_b(b, X, scr):
            """Tiny k/scores/softmax chain, channel-mix, DMA-out."""
            # kT[t, d] = sum_j s[(j,d), t]  (s lives in scr[:, :T] after the tree)
            kT_ps = psmall.tile([T, C], fp32, tag="ps")
            nc.tensor.matmul(
                kT_ps[:], lhsT=scr[:, :T], rhs=sel_t[:], start=True, stop=True
            )
            kT_sb = spool.tile([T, C], fp32, tag="kTsb")
            nc.scalar.copy(kT_sb[:], kT_ps[:])

            # V[t, d] = sum_s Wc[t, s] k[d, s]
            v_ps = psmall.tile([T, C], fp32, tag="ps")
            nc.tensor.matmul(v_ps[:], lhsT=wcT_t[:], rhs=kT_sb[:], start=True, stop=True)
            v_sb = spool.tile([T, C], fp32, tag="vsb")
            nc.scalar.copy(v_sb[:], v_ps[:])

            # scores[c, d] = sum_t k[c, t] V[t, d]
            sc_ps = psmall.tile([C, C], fp32, tag="ps")
            nc.tensor.matmul(sc_ps[:], lhsT=kT_sb[:], rhs=v_sb[:], start=True, stop=True)

            # unnormalized softmax: e = exp(scores), ssum = row sums
            # (scores for this problem are bounded ~|100|: exp stays in fp32
            # range; normalization happens at PSUM evacuation)
            e_sb = spool.tile([C, C], fp32, tag="esb")
            ssum = spool.tile([C, 1], fp32, tag="ssum")
            nc.scalar.activation(e_sb[:], sc_ps[:], Exp, accum_out=ssum[:])
            rcp = spool.tile([C, 1], fp32, tag="rcp")
            nc.vector.reciprocal(rcp[:], ssum[:])

            # replicate 1/sum to mix-output partitions: rsum[(j,c), 1]
            rs_ps = psmall.tile([P, 1], fp32, tag="ps")
            nc.tensor.matmul(rs_ps[:], lhsT=rep_t[:], rhs=rcp[:], start=True, stop=True)
            rs_sb = spool.tile([P, 1], fp32, tag="rssb")
            nc.scalar.copy(rs_sb[:], rs_ps[:])

            # replicate e^T to all j-blocks: erep[(j,d), c] = e[c, d]
            eT_ps = psmall.tile([C, C], fp32, tag="ps")
            nc.tensor.transpose(eT_ps[:], e_sb[:], id8_t[:])
            eT_sb = spool.tile([C, C], fp32, tag="eTsb")
            nc.scalar.copy(eT_sb[:], eT_ps[:])
            er_ps = psmall.tile([P, C], fp32, tag="ps")
            nc.tensor.matmul(
                er_ps[:], lhsT=rep_t[:], rhs=eT_sb[:], start=True, stop=True
            )
            # bd[(j,d), (j',c)] = mask * erep  (block-diagonal e^T)
            bd = bdpool.tile([P, P], fp32, tag="bd")
            nc.vector.tensor_tensor(
                out=bd[:].rearrange("p (j c) -> p j c", j=J),
                in0=mask_t[:].rearrange("p (j c) -> p j c", j=J),
                in1=er_ps[:].rearrange("p (x c) -> p x c", x=1).to_broadcast([P, J, C]),
                op=MULT,
            )

            # channel mix + normalized evacuation, half-granular staging so
            # output slots recycle quickly and descriptors stay 16KB
            FQ = F // 2
            out_b = out[b].rearrange("c (j n2) t -> j c (n2 t)", j=J)
            for qs in range(2):
                ost = opool.tile([P, FQ], fp32, tag="ost")
                for qq in range(FQ // QW):
                    q = qs * (FQ // QW) + qq
                    mp = mixp.tile([P, QW], fp32, tag="mix")
                    nc.tensor.matmul(
                        mp[:], lhsT=bd[:], rhs=X[:, q * QW : (q + 1) * QW],
                        start=True, stop=True,
                    )
                    nc.scalar.activation(
                        ost[:, qq * QW : (qq + 1) * QW], mp[:], Copy, scale=rs_sb[:]
                    )
                # second HWDGE ring (ACT) so in/out streams issue in parallel
                nc.scalar.dma_start(
                    out_b[:, :, qs * FQ : (qs + 1) * FQ],
                    ost[:],
                )

        # strict per-batch emission: with scr bufs=1 the next batch's big DVE
        # multiply has to queue behind this batch's kT matmul anyway, and
        # keeping recip/bd ahead of it in the DVE queue lets the mix (and the
        # X-slot release) happen early
        for b in range(BS):
            phase_b(b, *phase_a(b))

    nc.compile()
    return nc


def _host_constants(Wc: np.ndarray, alpha: np.ndarray):
    # ac[(j*8+d), n2] = alpha[j*128+n2]  (independent of d)
    a = alpha.reshape(J, N2).astype(np.float32)          # [16, 128]
    ac = np.repeat(a, C, axis=0)                         # [128, 128]
    # sel[(j*8+d), d'] = 1 if d == d'
    sel = np.tile(np.eye(C, dtype=np.float32), (J, 1))
    id8 = np.eye(C, dtype=np.float32)
    # rep[c', j*8+c] = 1 if c == c'  (partition replication)
    rep = np.tile(np.eye(C, dtype=np.float32), (1, J))
    # mask[(j,d), (j',c)] = 1 if j == j'
    mask = np.kron(np.eye(J, dtype=np.float32), np.ones((C, C), dtype=np.float32))
    return {
        "wcT": np.ascontiguousarray(Wc.T, dtype=np.float32),
        "ac": np.ascontiguousarray(ac),
        "sel": np.ascontiguousarray(sel),
        "id8": id8,
        "rep": np.ascontiguousarray(rep),
        "mask": np.ascontiguousarray(mask),
    }


def get_program():
    if "nc" not in _PROGRAM_CACHE:
        _PROGRAM_CACHE["nc"] = _build_program()
    return _PROGRAM_CACHE["nc"]


def run(x, Wc, alpha, trace=False, trace_kwargs=None):
    """Run on 8 cores; returns (full_output, BassKernelResults)."""
    from concourse.bass_utils import run_bass_kernel_spmd

    nc = get_program()
    consts = _host_constants(np.asarray(Wc), np.asarray(alpha))
    x = np.asarray(x, dtype=np.float32)
    in_maps = []
    for r in range(NCORES):
        m = {"xs": np.ascontiguousarray(x[r * BS : (r + 1) * BS])}
        m.update(consts)
        in_maps.append(m)
    kw = {}
    if trace:
        kw["trace"] = True
        if trace_kwargs:
            kw.update(trace_kwargs)
    res = run_bass_kernel_spmd(nc, in_maps, list(range(NCORES)), **kw)
    out = np.concatenate([res.results[r]["out"] for r in range(NCORES)], axis=0)
    return out, res


def kernel(x, Wc, alpha):
    out, _ = run(x, Wc, alpha)
    return out.astype(np.float32)
